# revision 29
# baseline (speedup 1.0000x reference)
"""DETR-style Hungarian-matching loss on 8 Trainium2 NeuronCores.

Strategy (pure data parallel, B=32 sharded 4 samples/core):
  Device (per core): the O(B*S^2*D) pairwise weighted-L1 cost matrix
    [4, 256, 256] plus the per-element BCE log terms.
    - The outer difference w_d*(pred[i,d] - gt[j,d]) is produced on the
      TensorEngine as K=4 matmuls per d: pred/gt are split into bf16
      hi+lo pairs on the host (exact to ~2^-17 rel), and the +-w_d
      weights ride along as constant rows.  PSUM accumulates in fp32.
    - A single fused VectorEngine tensor_reduce(apply_absolute_value)
      per tile folds |.| and the sum over d.
    - BCE: ScalarEngine Ln(p) / Ln(1-p), blended with the 0/1 targets.
  Host: shard/pack inputs, exact Jonker-Volgenant assignment per sample
    on the device-produced cost (sequential, data-dependent control
    flow - unsuited to the engines), and the final tiny reduction.
"""

import numpy as np
import ml_dtypes

B, S, D = 32, 256, 10
NCORES = 8
BPC = B // NCORES  # samples per core
_W = np.array([5.0] * 8 + [1.0] * 2, dtype=np.float32)  # per-dim L1 weights

_NC_CACHE = {}


def _build_program():
    import concourse.tile as tile
    import concourse.mybir as mybir
    from concourse import bacc

    f32 = mybir.dt.float32
    bf16 = mybir.dt.bfloat16
    AX = mybir.AxisListType
    AL = mybir.AluOpType
    ACTF = mybir.ActivationFunctionType

    # Bacc (not bass.Bass): its compile() splits multi-sem waits so matmuls
    # satisfy the TRN2 one-wait-per-instruction codegen constraint.
    nc = bacc.Bacc("TRN2", target_bir_lowering=False, debug=False)

    # Host-packed input. PE matmul operands must sit at partition base
    # 0/32/64, so the K=4 rows live on partitions 0..3; (sample, side, dim)
    # are multiplexed along the free dim: sample s occupies [s*5120,
    # (s+1)*5120) with its lhsT block (pm) first and rhs block (gm) second,
    # each indexed d*256 + col.  One contiguous DMA per sample -> the
    # sample's matmuls carry a single DMA wait (walrus limits MM waits).
    #   pm rows: [p_hi[:,d]; p_lo[:,d]; -w_d; -w_d]   (lhsT, i on free)
    #   gm rows: [ w_d    ;  w_d    ; g_hi[:,d]; g_lo[:,d]]  (rhs, j on free)
    # => (lhsT.T @ rhs)[i, j] = w_d*(p_hi+p_lo)[i] - w_d*(g_hi+g_lo)[j]
    # Samples land on partition bases {0, 32, 64, 0(+free offset)} so the
    # input DMAs hit three SBUF port groups in parallel (a 4-partition-only
    # destination is port-serialized ~13GB/s).
    SAMP = 2 * D * S  # 5120
    S_BASE = [0, 32, 64, 0]
    S_FOFF = [0, 0, 0, SAMP]
    pg_d = nc.dram_tensor("pg", [BPC, 4, SAMP], bf16, kind="ExternalInput").ap()
    pv_d = nc.dram_tensor("pv", [128, 8], f32, kind="ExternalInput").ap()
    tv_d = nc.dram_tensor("tv", [128, 8], f32, kind="ExternalInput").ap()
    cost_d = nc.dram_tensor("cost", [BPC, S, S], f32, kind="ExternalOutput").ap()
    bce_d = nc.dram_tensor("bce", [128, 8], f32, kind="ExternalOutput").ap()

    with tile.TileContext(nc) as tc:
        with (
            tc.tile_pool(name="inp", bufs=1) as inp,
            tc.tile_pool(name="ps", bufs=2, space="PSUM") as psp,
            tc.tile_pool(name="outp", bufs=4) as outp,
            tc.tile_pool(name="bcep", bufs=1) as bcep,
        ):
            # Input DMAs: one chunk per sample, sample 0 first (it gates the
            # first matmuls; measured best all on the sync queue).
            pg_sb = inp.tile([68, 2 * SAMP], bf16, tag="pg", name="pg_sb")
            for s in range(BPC):
                b, f = S_BASE[s], S_FOFF[s]
                nc.sync.dma_start(
                    pg_sb[b : b + 4, f : f + SAMP],
                    pg_d[s],
                )
            pv_sb = bcep.tile([128, 8], f32, tag="pv")
            tv_sb = bcep.tile([128, 8], f32, tag="tv")
            nc.scalar.dma_start(pv_sb[:], pv_d[:])
            nc.scalar.dma_start(tv_sb[:], tv_d[:])

            # ---- cost matrix: 16 units of [128 i, 10 d, 128 j] ----
            # The DVE fused abs-reduce is the kernel bottleneck (~1.46us/unit,
            # saturated).  Offload every 4th unit to the otherwise-idle
            # ScalarE (Abs fused into the PSUM->SBUF copy) + GpSimd (the
            # d-sum as 4 strided tensor_tensor adds), shortening the DVE
            # critical path.
            uidx = 0
            for s in range(BPC):
                for it in range(2):
                    for jh in range(2):
                        ps = psp.tile([128, 10, 128], f32, tag="unit")
                        b, f = S_BASE[s], S_FOFF[s]
                        for d in range(10):
                            fp = f + d * S
                            fg = f + D * S + d * S
                            nc.tensor.matmul(
                                ps[:, d, :],
                                pg_sb[b : b + 4, fp + 128 * it : fp + 128 * (it + 1)],
                                pg_sb[b : b + 4, fg + 128 * jh : fg + 128 * (jh + 1)],
                                start=True,
                                stop=True,
                            )
                        dst = cost_d[s, 128 * it : 128 * (it + 1), 128 * jh : 128 * (jh + 1)]
                        if uidx in (1, 3, 5, 7, 9, 11):
                            # Offload: ACT folds |.| into the PSUM->SBUF copy,
                            # GpSimd halves the planes (10->5), DVE finishes
                            # with a cheap SBUF-side 5-plane reduce.
                            absb = outp.tile([128, 10, 128], f32, tag="absb", bufs=2)
                            nc.scalar.activation(absb[:], ps[:], ACTF.Abs)
                            t5 = outp.tile([128, 5, 128], f32, tag="t5", bufs=2)
                            nc.gpsimd.tensor_tensor(
                                t5[:], absb[:, 0:9:2, :], absb[:, 1:10:2, :], op=AL.add
                            )
                            cg = outp.tile([128, 128], f32, tag="cg", bufs=2)
                            nc.vector.tensor_reduce(
                                cg[:],
                                t5[:].rearrange("p d j -> p j d"),
                                axis=AX.X,
                                op=AL.add,
                            )
                            nc.sync.dma_start(dst, cg[:])
                        else:
                            csb = outp.tile([128, 128], f32, tag="csb")
                            nc.vector.tensor_reduce(
                                csb[:],
                                ps[:].rearrange("p d j -> p j d"),
                                axis=AX.X,
                                op=AL.add,
                                apply_absolute_value=True,
                            )
                            nc.sync.dma_start(dst, csb[:])
                        uidx += 1

            # ---- BCE log terms (tiny): term = t*ln(p) + (1-t)*ln(1-p) ----
            lnp = bcep.tile([128, 8], f32, tag="lnp")
            ln1p = bcep.tile([128, 8], f32, tag="ln1p")
            nc.scalar.activation(lnp[:], pv_sb[:], ACTF.Ln)
            nc.scalar.activation(ln1p[:], pv_sb[:], ACTF.Ln, scale=-1.0, bias=1.0)
            dif = bcep.tile([128, 8], f32, tag="dif")
            nc.vector.tensor_tensor(dif[:], lnp[:], ln1p[:], op=AL.subtract)
            prod = bcep.tile([128, 8], f32, tag="prod")
            nc.vector.tensor_tensor(prod[:], dif[:], tv_sb[:], op=AL.mult)
            term = bcep.tile([128, 8], f32, tag="term")
            nc.vector.tensor_tensor(term[:], prod[:], ln1p[:], op=AL.add)
            nc.sync.dma_start(bce_d[:], term[:])
    nc.compile()
    return nc


def _get_nc():
    if "nc" not in _NC_CACHE:
        _NC_CACHE["nc"] = _build_program()
    return _NC_CACHE["nc"]


def _pack_core(ps_c, pv_c, tg_c):
    """Build the per-core input map from fp32 slices [BPC,S,10/1/11]."""
    bf = ml_dtypes.bfloat16
    gt_c = tg_c[..., :10]
    p_hi = ps_c.astype(bf)
    p_lo = (ps_c - p_hi.astype(np.float32)).astype(bf)
    g_hi = gt_c.astype(bf)
    g_lo = (gt_c - g_hi.astype(np.float32)).astype(bf)
    w_b = _W.astype(bf)

    pg = np.zeros((BPC, 4, 2, D, S), dtype=bf)
    ones = np.ones((BPC, D, S), dtype=np.float32)
    pg[:, 0, 0] = p_hi.transpose(0, 2, 1)
    pg[:, 1, 0] = p_lo.transpose(0, 2, 1)
    pg[:, 2, 0] = -w_b[None, :, None] * ones
    pg[:, 3, 0] = pg[:, 2, 0]
    pg[:, 0, 1] = w_b[None, :, None] * ones
    pg[:, 1, 1] = pg[:, 0, 1]
    pg[:, 2, 1] = g_hi.transpose(0, 2, 1)
    pg[:, 3, 1] = g_lo.transpose(0, 2, 1)
    pg = np.ascontiguousarray(pg.reshape(BPC, 4, 2 * D * S))

    pv = np.ascontiguousarray(pv_c.reshape(1024).reshape(128, 8))
    tv = np.ascontiguousarray(tg_c[..., 10].reshape(1024).reshape(128, 8))
    return {"pg": pg, "pv": pv, "tv": tv}


def _lsa(cost):
    """Rectangular linear sum assignment (Jonker-Volgenant shortest augmenting
    path), equivalent to scipy.optimize.linear_sum_assignment."""
    cost = np.asarray(cost, dtype=np.float64)
    transposed = cost.shape[0] > cost.shape[1]
    if transposed:
        cost = cost.T
    nr, ncc = cost.shape
    u = np.zeros(nr)
    v = np.zeros(ncc)
    path = np.full(ncc, -1, dtype=np.int64)
    col4row = np.full(nr, -1, dtype=np.int64)
    row4col = np.full(ncc, -1, dtype=np.int64)
    for cur_row in range(nr):
        min_val = 0.0
        i = cur_row
        remaining = np.arange(ncc)
        sp = np.full(ncc, np.inf)
        SR = np.zeros(nr, dtype=bool)
        SC = np.zeros(ncc, dtype=bool)
        sink = -1
        while sink == -1:
            SR[i] = True
            r = min_val + cost[i, remaining] - u[i] - v[remaining]
            better = r < sp[remaining]
            idx_b = remaining[better]
            sp[idx_b] = r[better]
            path[idx_b] = i
            k = int(np.argmin(sp[remaining]))
            j = int(remaining[k])
            min_val = float(sp[j])
            if row4col[j] == -1:
                sink = j
            else:
                i = int(row4col[j])
                SC[j] = True
                remaining = np.delete(remaining, k)
        u[cur_row] += min_val
        others = SR.copy()
        others[cur_row] = False
        rr = np.where(others)[0]
        u[rr] += min_val - sp[col4row[rr]]
        cc = np.where(SC)[0]
        v[cc] += sp[cc] - min_val
        j = sink
        while True:
            i = int(path[j])
            row4col[j] = i
            col4row[i], j = j, int(col4row[i])
            if i == cur_row:
                break
    if transposed:
        rows = col4row
        cols = np.arange(nr)
        order = np.argsort(rows)
        return rows[order], cols[order]
    return np.arange(nr), col4row


def _assemble_loss(pred_strokes, targets, cost, bce_terms):
    """Host: per-sample Hungarian on the device cost + final reduction."""
    gt = targets[..., :10]
    gval = targets[..., 10]
    total = 0.0
    for b in range(B):
        bce_b = -float(
            bce_terms[b // BPC].reshape(1024)[
                (b % BPC) * 256 : (b % BPC + 1) * 256
            ].astype(np.float64).mean()
        )
        cols = np.where(gval[b] > 0.5)[0]
        n = len(cols)
        if n == 0:
            total += bce_b
            continue
        r, c = _lsa(cost[b][:, cols])
        pi, gi = r, cols[c]
        mp = pred_strokes[b, pi].astype(np.float64)
        mg = gt[b, gi].astype(np.float64)
        coord = np.abs(mp[:, :8] - mg[:, :8]).sum() / max(8 * n, 1)
        width = np.abs(mp[:, 8:10] - mg[:, 8:10]).sum() / max(2 * n, 1)
        total += 5.0 * coord + width + bce_b
    return np.float32(total / B)


def kernel(pred_strokes, pred_validity, targets):
    from concourse.bass_utils import run_bass_kernel_spmd

    pred_strokes = np.asarray(pred_strokes, dtype=np.float32)
    pred_validity = np.asarray(pred_validity, dtype=np.float32)
    targets = np.asarray(targets, dtype=np.float32)

    in_maps = [
        _pack_core(
            pred_strokes[c * BPC : (c + 1) * BPC],
            pred_validity[c * BPC : (c + 1) * BPC],
            targets[c * BPC : (c + 1) * BPC],
        )
        for c in range(NCORES)
    ]

    nc = _get_nc()
    res = run_bass_kernel_spmd(nc, in_maps, list(range(NCORES)))
    results = res.results

    cost = np.concatenate([r["cost"] for r in results], axis=0)  # [32, 256, 256]
    bce_terms = [r["bce"] for r in results]  # 8 x [128, 8]

    loss = _assemble_loss(pred_strokes, targets, cost, bce_terms)
    return np.asarray(loss, dtype=np.float32)


# revision 33
# speedup vs baseline: 1.1314x; 1.1314x over previous
"""DETR-style Hungarian-matching loss on 8 Trainium2 NeuronCores.

Strategy (pure data parallel, B=32 sharded 4 samples/core):
  Device (per core): the O(B*S^2*D) pairwise weighted-L1 cost matrix
    [4, 256, 256] plus the per-element BCE log terms.
    - The outer difference w_d*(pred[i,d] - gt[j,d]) is produced on the
      TensorEngine as K=4 matmuls per d: pred/gt are split into bf16
      hi+lo pairs on the host (exact to ~2^-17 rel), and the +-w_d
      weights ride along as constant rows.  PSUM accumulates in fp32.
    - A single fused VectorEngine tensor_reduce(apply_absolute_value)
      per tile folds |.| and the sum over d.
    - BCE: ScalarEngine Ln(p) / Ln(1-p), blended with the 0/1 targets.
  Host: shard/pack inputs, exact Jonker-Volgenant assignment per sample
    on the device-produced cost (sequential, data-dependent control
    flow - unsuited to the engines), and the final tiny reduction.
"""

import numpy as np
import ml_dtypes

B, S, D = 32, 256, 10
NCORES = 8
BPC = B // NCORES  # samples per core
_W = np.array([5.0] * 8 + [1.0] * 2, dtype=np.float32)  # per-dim L1 weights

_NC_CACHE = {}


def _build_program():
    import concourse.tile as tile
    import concourse.mybir as mybir
    from concourse import bacc

    f32 = mybir.dt.float32
    bf16 = mybir.dt.bfloat16
    AX = mybir.AxisListType
    AL = mybir.AluOpType
    ACTF = mybir.ActivationFunctionType

    # Bacc (not bass.Bass): its compile() splits multi-sem waits so matmuls
    # satisfy the TRN2 one-wait-per-instruction codegen constraint.
    nc = bacc.Bacc("TRN2", target_bir_lowering=False, debug=False)

    # Host-packed input. PE matmul operands must sit at partition base
    # 0/32/64, so the K=4 rows live on partitions 0..3; (sample, side, dim)
    # are multiplexed along the free dim: sample s occupies [s*5120,
    # (s+1)*5120) with its lhsT block (pm) first and rhs block (gm) second,
    # each indexed d*256 + col.  One contiguous DMA per sample -> the
    # sample's matmuls carry a single DMA wait (walrus limits MM waits).
    #   pm rows: [p_hi[:,d]; p_lo[:,d]; -w_d; -w_d]   (lhsT, i on free)
    #   gm rows: [ w_d    ;  w_d    ; g_hi[:,d]; g_lo[:,d]]  (rhs, j on free)
    # => (lhsT.T @ rhs)[i, j] = w_d*(p_hi+p_lo)[i] - w_d*(g_hi+g_lo)[j]
    # Samples land on partition bases {0, 32, 64, 0(+free offset)} so the
    # input DMAs hit three SBUF port groups in parallel (a 4-partition-only
    # destination is port-serialized ~13GB/s).
    SAMP = 2 * D * S  # 5120
    S_BASE = [0, 32, 64, 0]
    S_FOFF = [0, 0, 0, SAMP]
    pg_d = nc.dram_tensor("pg", [BPC, 4, SAMP], bf16, kind="ExternalInput").ap()
    pv_d = nc.dram_tensor("pv", [128, 8], f32, kind="ExternalInput").ap()
    tv_d = nc.dram_tensor("tv", [128, 8], f32, kind="ExternalInput").ap()
    cost_d = nc.dram_tensor("cost", [BPC, S, S], f32, kind="ExternalOutput").ap()
    bce_d = nc.dram_tensor("bce", [128, 8], f32, kind="ExternalOutput").ap()

    with tile.TileContext(nc) as tc:
        with (
            tc.tile_pool(name="inp", bufs=1) as inp,
            tc.tile_pool(name="ps", bufs=2, space="PSUM") as psp,
            tc.tile_pool(name="outp", bufs=4) as outp,
            tc.tile_pool(name="bcep", bufs=1) as bcep,
        ):
            # Input DMAs: one chunk per sample, sample 0 first (it gates the
            # first matmuls; measured best all on the sync queue).
            pg_sb = inp.tile([68, 2 * SAMP], bf16, tag="pg", name="pg_sb")
            for s in range(BPC):
                b, f = S_BASE[s], S_FOFF[s]
                if s == 0:
                    # Sample 0 gates the first matmuls: land the (it=0, jh=0)
                    # column halves first so unit 0 can start sooner.
                    dst = pg_sb[b : b + 4, f : f + SAMP].rearrange(
                        "p (k c) -> p k c", c=S
                    )
                    src = pg_d[s].rearrange("p (k c) -> p k c", c=S)
                    nc.sync.dma_start(dst[:, :, 0:128], src[:, :, 0:128])
                    nc.sync.dma_start(dst[:, :, 128:256], src[:, :, 128:256])
                else:
                    nc.sync.dma_start(
                        pg_sb[b : b + 4, f : f + SAMP],
                        pg_d[s],
                    )
            pv_sb = bcep.tile([128, 8], f32, tag="pv")
            tv_sb = bcep.tile([128, 8], f32, tag="tv")
            nc.scalar.dma_start(pv_sb[:], pv_d[:])
            nc.scalar.dma_start(tv_sb[:], tv_d[:])

            # ---- cost matrix: 16 units of [128 i, 10 d, 128 j] ----
            for s in range(BPC):
                for it in range(2):
                    for jh in range(2):
                        ps = psp.tile([128, 10, 128], f32, tag="unit")
                        b, f = S_BASE[s], S_FOFF[s]
                        for d in range(10):
                            fp = f + d * S
                            fg = f + D * S + d * S
                            nc.tensor.matmul(
                                ps[:, d, :],
                                pg_sb[b : b + 4, fp + 128 * it : fp + 128 * (it + 1)],
                                pg_sb[b : b + 4, fg + 128 * jh : fg + 128 * (jh + 1)],
                                start=True,
                                stop=True,
                            )
                        csb = outp.tile([128, 128], f32, tag="csb")
                        nc.vector.tensor_reduce(
                            csb[:],
                            ps[:].rearrange("p d j -> p j d"),
                            axis=AX.X,
                            op=AL.add,
                            apply_absolute_value=True,
                        )
                        nc.sync.dma_start(
                            cost_d[s, 128 * it : 128 * (it + 1), 128 * jh : 128 * (jh + 1)],
                            csb[:],
                        )

            # ---- BCE log terms (tiny): term = t*ln(p) + (1-t)*ln(1-p) ----
            lnp = bcep.tile([128, 8], f32, tag="lnp")
            ln1p = bcep.tile([128, 8], f32, tag="ln1p")
            nc.scalar.activation(lnp[:], pv_sb[:], ACTF.Ln)
            nc.scalar.activation(ln1p[:], pv_sb[:], ACTF.Ln, scale=-1.0, bias=1.0)
            dif = bcep.tile([128, 8], f32, tag="dif")
            nc.vector.tensor_tensor(dif[:], lnp[:], ln1p[:], op=AL.subtract)
            prod = bcep.tile([128, 8], f32, tag="prod")
            nc.vector.tensor_tensor(prod[:], dif[:], tv_sb[:], op=AL.mult)
            term = bcep.tile([128, 8], f32, tag="term")
            nc.vector.tensor_tensor(term[:], prod[:], ln1p[:], op=AL.add)
            nc.sync.dma_start(bce_d[:], term[:])
    nc.compile()
    return nc


def _get_nc():
    if "nc" not in _NC_CACHE:
        _NC_CACHE["nc"] = _build_program()
    return _NC_CACHE["nc"]


def _pack_core(ps_c, pv_c, tg_c):
    """Build the per-core input map from fp32 slices [BPC,S,10/1/11]."""
    bf = ml_dtypes.bfloat16
    gt_c = tg_c[..., :10]
    p_hi = ps_c.astype(bf)
    p_lo = (ps_c - p_hi.astype(np.float32)).astype(bf)
    g_hi = gt_c.astype(bf)
    g_lo = (gt_c - g_hi.astype(np.float32)).astype(bf)
    w_b = _W.astype(bf)

    pg = np.zeros((BPC, 4, 2, D, S), dtype=bf)
    ones = np.ones((BPC, D, S), dtype=np.float32)
    pg[:, 0, 0] = p_hi.transpose(0, 2, 1)
    pg[:, 1, 0] = p_lo.transpose(0, 2, 1)
    pg[:, 2, 0] = -w_b[None, :, None] * ones
    pg[:, 3, 0] = pg[:, 2, 0]
    pg[:, 0, 1] = w_b[None, :, None] * ones
    pg[:, 1, 1] = pg[:, 0, 1]
    pg[:, 2, 1] = g_hi.transpose(0, 2, 1)
    pg[:, 3, 1] = g_lo.transpose(0, 2, 1)
    pg = np.ascontiguousarray(pg.reshape(BPC, 4, 2 * D * S))

    pv = np.ascontiguousarray(pv_c.reshape(1024).reshape(128, 8))
    tv = np.ascontiguousarray(tg_c[..., 10].reshape(1024).reshape(128, 8))
    return {"pg": pg, "pv": pv, "tv": tv}


def _lsa(cost):
    """Rectangular linear sum assignment (Jonker-Volgenant shortest augmenting
    path), equivalent to scipy.optimize.linear_sum_assignment."""
    cost = np.asarray(cost, dtype=np.float64)
    transposed = cost.shape[0] > cost.shape[1]
    if transposed:
        cost = cost.T
    nr, ncc = cost.shape
    u = np.zeros(nr)
    v = np.zeros(ncc)
    path = np.full(ncc, -1, dtype=np.int64)
    col4row = np.full(nr, -1, dtype=np.int64)
    row4col = np.full(ncc, -1, dtype=np.int64)
    for cur_row in range(nr):
        min_val = 0.0
        i = cur_row
        remaining = np.arange(ncc)
        sp = np.full(ncc, np.inf)
        SR = np.zeros(nr, dtype=bool)
        SC = np.zeros(ncc, dtype=bool)
        sink = -1
        while sink == -1:
            SR[i] = True
            r = min_val + cost[i, remaining] - u[i] - v[remaining]
            better = r < sp[remaining]
            idx_b = remaining[better]
            sp[idx_b] = r[better]
            path[idx_b] = i
            k = int(np.argmin(sp[remaining]))
            j = int(remaining[k])
            min_val = float(sp[j])
            if row4col[j] == -1:
                sink = j
            else:
                i = int(row4col[j])
                SC[j] = True
                remaining = np.delete(remaining, k)
        u[cur_row] += min_val
        others = SR.copy()
        others[cur_row] = False
        rr = np.where(others)[0]
        u[rr] += min_val - sp[col4row[rr]]
        cc = np.where(SC)[0]
        v[cc] += sp[cc] - min_val
        j = sink
        while True:
            i = int(path[j])
            row4col[j] = i
            col4row[i], j = j, int(col4row[i])
            if i == cur_row:
                break
    if transposed:
        rows = col4row
        cols = np.arange(nr)
        order = np.argsort(rows)
        return rows[order], cols[order]
    return np.arange(nr), col4row


def _assemble_loss(pred_strokes, targets, cost, bce_terms):
    """Host: per-sample Hungarian on the device cost + final reduction."""
    gt = targets[..., :10]
    gval = targets[..., 10]
    total = 0.0
    for b in range(B):
        bce_b = -float(
            bce_terms[b // BPC].reshape(1024)[
                (b % BPC) * 256 : (b % BPC + 1) * 256
            ].astype(np.float64).mean()
        )
        cols = np.where(gval[b] > 0.5)[0]
        n = len(cols)
        if n == 0:
            total += bce_b
            continue
        r, c = _lsa(cost[b][:, cols])
        pi, gi = r, cols[c]
        mp = pred_strokes[b, pi].astype(np.float64)
        mg = gt[b, gi].astype(np.float64)
        coord = np.abs(mp[:, :8] - mg[:, :8]).sum() / max(8 * n, 1)
        width = np.abs(mp[:, 8:10] - mg[:, 8:10]).sum() / max(2 * n, 1)
        total += 5.0 * coord + width + bce_b
    return np.float32(total / B)


def kernel(pred_strokes, pred_validity, targets):
    from concourse.bass_utils import run_bass_kernel_spmd

    pred_strokes = np.asarray(pred_strokes, dtype=np.float32)
    pred_validity = np.asarray(pred_validity, dtype=np.float32)
    targets = np.asarray(targets, dtype=np.float32)

    in_maps = [
        _pack_core(
            pred_strokes[c * BPC : (c + 1) * BPC],
            pred_validity[c * BPC : (c + 1) * BPC],
            targets[c * BPC : (c + 1) * BPC],
        )
        for c in range(NCORES)
    ]

    nc = _get_nc()
    res = run_bass_kernel_spmd(nc, in_maps, list(range(NCORES)))
    results = res.results

    cost = np.concatenate([r["cost"] for r in results], axis=0)  # [32, 256, 256]
    bce_terms = [r["bce"] for r in results]  # 8 x [128, 8]

    loss = _assemble_loss(pred_strokes, targets, cost, bce_terms)
    return np.asarray(loss, dtype=np.float32)


# revision 37
# speedup vs baseline: 1.1362x; 1.0043x over previous
"""DETR-style Hungarian-matching loss on 8 Trainium2 NeuronCores.

Strategy (pure data parallel, B=32 sharded 4 samples/core):
  Device (per core): the O(B*S^2*D) pairwise weighted-L1 cost matrix
    [4, 256, 256] plus the per-element BCE log terms.
    - The outer difference w_d*(pred[i,d] - gt[j,d]) is produced on the
      TensorEngine as K=4 matmuls per d: pred/gt are split into bf16
      hi+lo pairs on the host (exact to ~2^-17 rel), and the +-w_d
      weights ride along as constant rows.  PSUM accumulates in fp32.
    - A single fused VectorEngine tensor_reduce(apply_absolute_value)
      per tile folds |.| and the sum over d.
    - BCE: ScalarEngine Ln(p) / Ln(1-p), blended with the 0/1 targets.
  Host: shard/pack inputs, exact Jonker-Volgenant assignment per sample
    on the device-produced cost (sequential, data-dependent control
    flow - unsuited to the engines), and the final tiny reduction.
"""

import numpy as np
import ml_dtypes

B, S, D = 32, 256, 10
NCORES = 8
BPC = B // NCORES  # samples per core
_W = np.array([5.0] * 8 + [1.0] * 2, dtype=np.float32)  # per-dim L1 weights

_NC_CACHE = {}


def _build_program():
    import concourse.tile as tile
    import concourse.mybir as mybir
    from concourse import bacc

    f32 = mybir.dt.float32
    bf16 = mybir.dt.bfloat16
    AX = mybir.AxisListType
    AL = mybir.AluOpType
    ACTF = mybir.ActivationFunctionType

    # Bacc (not bass.Bass): its compile() splits multi-sem waits so matmuls
    # satisfy the TRN2 one-wait-per-instruction codegen constraint.
    nc = bacc.Bacc("TRN2", target_bir_lowering=False, debug=False)

    # Host-packed input. PE matmul operands must sit at partition base
    # 0/32/64, so the K=4 rows live on partitions 0..3; (sample, side, dim)
    # are multiplexed along the free dim: sample s occupies [s*5120,
    # (s+1)*5120) with its lhsT block (pm) first and rhs block (gm) second,
    # each indexed d*256 + col.  One contiguous DMA per sample -> the
    # sample's matmuls carry a single DMA wait (walrus limits MM waits).
    #   pm rows: [p_hi[:,d]; p_lo[:,d]; -w_d; -w_d]   (lhsT, i on free)
    #   gm rows: [ w_d    ;  w_d    ; g_hi[:,d]; g_lo[:,d]]  (rhs, j on free)
    # => (lhsT.T @ rhs)[i, j] = w_d*(p_hi+p_lo)[i] - w_d*(g_hi+g_lo)[j]
    # Samples land on partition bases {0, 32, 64, 0(+free offset)} so the
    # input DMAs hit three SBUF port groups in parallel (a 4-partition-only
    # destination is port-serialized ~13GB/s).
    SAMP = 2 * D * S  # 5120
    S_BASE = [0, 32, 64, 0]
    S_FOFF = [0, 0, 0, SAMP]
    pg_d = nc.dram_tensor("pg", [BPC, 4, SAMP], bf16, kind="ExternalInput").ap()
    pv_d = nc.dram_tensor("pv", [128, 8], f32, kind="ExternalInput").ap()
    tv_d = nc.dram_tensor("tv", [128, 8], f32, kind="ExternalInput").ap()
    cost_d = nc.dram_tensor("cost", [BPC, S, S], f32, kind="ExternalOutput").ap()
    bce_d = nc.dram_tensor("bce", [128, 8], f32, kind="ExternalOutput").ap()

    with tile.TileContext(nc) as tc:
        with (
            tc.tile_pool(name="inp", bufs=1) as inp,
            tc.tile_pool(name="ps", bufs=2, space="PSUM") as psp,
            tc.tile_pool(name="outp", bufs=4) as outp,
            tc.tile_pool(name="bcep", bufs=1) as bcep,
        ):
            # Input DMAs: one chunk per sample, sample 0 first (it gates the
            # first matmuls; measured best all on the sync queue).
            pg_sb = inp.tile([68, 2 * SAMP], bf16, tag="pg", name="pg_sb")
            for s in range(BPC):
                b, f = S_BASE[s], S_FOFF[s]
                if s == 0:
                    # Sample 0 gates the first matmuls: land the (it=0, jh=0)
                    # column halves first so unit 0 can start sooner.
                    dst = pg_sb[b : b + 4, f : f + SAMP].rearrange(
                        "p (k c) -> p k c", c=S
                    )
                    src = pg_d[s].rearrange("p (k c) -> p k c", c=S)
                    nc.sync.dma_start(dst[:, :, 0:128], src[:, :, 0:128])
                    nc.sync.dma_start(dst[:, :, 128:256], src[:, :, 128:256])
                else:
                    nc.sync.dma_start(
                        pg_sb[b : b + 4, f : f + SAMP],
                        pg_d[s],
                    )
            pv_sb = bcep.tile([128, 8], f32, tag="pv")
            tv_sb = bcep.tile([128, 8], f32, tag="tv")
            nc.scalar.dma_start(pv_sb[:], pv_d[:])
            nc.scalar.dma_start(tv_sb[:], tv_d[:])

            # ---- cost matrix: 16 units of [128 i, 10 d, 128 j] ----
            for s in range(BPC):
                for it in range(2):
                    for jh in range(2):
                        ps = psp.tile([128, 10, 128], f32, tag="unit")
                        b, f = S_BASE[s], S_FOFF[s]
                        for d in range(10):
                            fp = f + d * S
                            fg = f + D * S + d * S
                            nc.tensor.matmul(
                                ps[:, d, :],
                                pg_sb[b : b + 4, fp + 128 * it : fp + 128 * (it + 1)],
                                pg_sb[b : b + 4, fg + 128 * jh : fg + 128 * (jh + 1)],
                                start=True,
                                stop=True,
                            )
                        csb = outp.tile([128, 128], f32, tag="csb")
                        nc.vector.tensor_reduce(
                            csb[:],
                            ps[:].rearrange("p d j -> p j d"),
                            axis=AX.X,
                            op=AL.add,
                            apply_absolute_value=True,
                        )
                        nc.sync.dma_start(
                            cost_d[s, 128 * it : 128 * (it + 1), 128 * jh : 128 * (jh + 1)],
                            csb[:],
                        )

            # ---- BCE log terms (tiny): term = t*ln(p) + (1-t)*ln(1-p) ----
            lnp = bcep.tile([128, 8], f32, tag="lnp")
            ln1p = bcep.tile([128, 8], f32, tag="ln1p")
            nc.scalar.activation(lnp[:], pv_sb[:], ACTF.Ln)
            nc.scalar.activation(ln1p[:], pv_sb[:], ACTF.Ln, scale=-1.0, bias=1.0)
            dif = bcep.tile([128, 8], f32, tag="dif")
            nc.vector.tensor_tensor(dif[:], lnp[:], ln1p[:], op=AL.subtract)
            prod = bcep.tile([128, 8], f32, tag="prod")
            nc.vector.tensor_tensor(prod[:], dif[:], tv_sb[:], op=AL.mult)
            term = bcep.tile([128, 8], f32, tag="term")
            nc.vector.tensor_tensor(term[:], prod[:], ln1p[:], op=AL.add)
            nc.sync.dma_start(bce_d[:], term[:])
    nc.compile()
    return nc


def _get_nc():
    if "nc" not in _NC_CACHE:
        _NC_CACHE["nc"] = _build_program()
    return _NC_CACHE["nc"]


def _pack_core(ps_c, pv_c, tg_c):
    """Build the per-core input map from fp32 slices [BPC,S,10/1/11]."""
    bf = ml_dtypes.bfloat16
    gt_c = tg_c[..., :10]
    p_hi = ps_c.astype(bf)
    p_lo = (ps_c - p_hi.astype(np.float32)).astype(bf)
    g_hi = gt_c.astype(bf)
    g_lo = (gt_c - g_hi.astype(np.float32)).astype(bf)
    w_b = _W.astype(bf)

    pg = np.zeros((BPC, 4, 2, D, S), dtype=bf)
    ones = np.ones((BPC, D, S), dtype=np.float32)
    pg[:, 0, 0] = p_hi.transpose(0, 2, 1)
    pg[:, 1, 0] = p_lo.transpose(0, 2, 1)
    pg[:, 2, 0] = -w_b[None, :, None] * ones
    pg[:, 3, 0] = pg[:, 2, 0]
    pg[:, 0, 1] = w_b[None, :, None] * ones
    pg[:, 1, 1] = pg[:, 0, 1]
    pg[:, 2, 1] = g_hi.transpose(0, 2, 1)
    pg[:, 3, 1] = g_lo.transpose(0, 2, 1)
    pg = np.ascontiguousarray(pg.reshape(BPC, 4, 2 * D * S))

    pv = np.ascontiguousarray(pv_c.reshape(1024).reshape(128, 8))
    tv = np.ascontiguousarray(tg_c[..., 10].reshape(1024).reshape(128, 8))
    return {"pg": pg, "pv": pv, "tv": tv}


def _lsa(cost):
    """Rectangular linear sum assignment (Jonker-Volgenant shortest augmenting
    path), equivalent to scipy.optimize.linear_sum_assignment."""
    cost = np.asarray(cost, dtype=np.float64)
    transposed = cost.shape[0] > cost.shape[1]
    if transposed:
        cost = cost.T
    nr, ncc = cost.shape
    u = np.zeros(nr)
    v = np.zeros(ncc)
    path = np.full(ncc, -1, dtype=np.int64)
    col4row = np.full(nr, -1, dtype=np.int64)
    row4col = np.full(ncc, -1, dtype=np.int64)
    for cur_row in range(nr):
        min_val = 0.0
        i = cur_row
        remaining = np.arange(ncc)
        sp = np.full(ncc, np.inf)
        SR = np.zeros(nr, dtype=bool)
        SC = np.zeros(ncc, dtype=bool)
        sink = -1
        while sink == -1:
            SR[i] = True
            r = min_val + cost[i, remaining] - u[i] - v[remaining]
            better = r < sp[remaining]
            idx_b = remaining[better]
            sp[idx_b] = r[better]
            path[idx_b] = i
            k = int(np.argmin(sp[remaining]))
            j = int(remaining[k])
            min_val = float(sp[j])
            if row4col[j] == -1:
                sink = j
            else:
                i = int(row4col[j])
                SC[j] = True
                remaining = np.delete(remaining, k)
        u[cur_row] += min_val
        others = SR.copy()
        others[cur_row] = False
        rr = np.where(others)[0]
        u[rr] += min_val - sp[col4row[rr]]
        cc = np.where(SC)[0]
        v[cc] += sp[cc] - min_val
        j = sink
        while True:
            i = int(path[j])
            row4col[j] = i
            col4row[i], j = j, int(col4row[i])
            if i == cur_row:
                break
    if transposed:
        rows = col4row
        cols = np.arange(nr)
        order = np.argsort(rows)
        return rows[order], cols[order]
    return np.arange(nr), col4row


def _assemble_loss(pred_strokes, targets, cost, bce_terms):
    """Host: per-sample Hungarian on the device cost + final reduction."""
    gt = targets[..., :10]
    gval = targets[..., 10]
    total = 0.0
    for b in range(B):
        bce_b = -float(
            bce_terms[b // BPC].reshape(1024)[
                (b % BPC) * 256 : (b % BPC + 1) * 256
            ].astype(np.float64).mean()
        )
        cols = np.where(gval[b] > 0.5)[0]
        n = len(cols)
        if n == 0:
            total += bce_b
            continue
        r, c = _lsa(cost[b][:, cols])
        pi, gi = r, cols[c]
        mp = pred_strokes[b, pi].astype(np.float64)
        mg = gt[b, gi].astype(np.float64)
        coord = np.abs(mp[:, :8] - mg[:, :8]).sum() / max(8 * n, 1)
        width = np.abs(mp[:, 8:10] - mg[:, 8:10]).sum() / max(2 * n, 1)
        total += 5.0 * coord + width + bce_b
    return np.float32(total / B)


def kernel(pred_strokes, pred_validity, targets):
    from concourse.bass_utils import run_bass_kernel_spmd

    pred_strokes = np.asarray(pred_strokes, dtype=np.float32)
    pred_validity = np.asarray(pred_validity, dtype=np.float32)
    targets = np.asarray(targets, dtype=np.float32)

    in_maps = [
        _pack_core(
            pred_strokes[c * BPC : (c + 1) * BPC],
            pred_validity[c * BPC : (c + 1) * BPC],
            targets[c * BPC : (c + 1) * BPC],
        )
        for c in range(NCORES)
    ]

    nc = _get_nc()
    res = run_bass_kernel_spmd(nc, in_maps, list(range(NCORES)))
    results = res.results

    cost = np.concatenate([r["cost"] for r in results], axis=0)  # [32, 256, 256]
    bce_terms = [r["bce"] for r in results]  # 8 x [128, 8]

    loss = _assemble_loss(pred_strokes, targets, cost, bce_terms)
    return np.asarray(loss, dtype=np.float32)


# revision 41
# speedup vs baseline: 1.1441x; 1.0070x over previous
"""DETR-style Hungarian-matching loss on 8 Trainium2 NeuronCores.

Strategy (pure data parallel, B=32 sharded 4 samples/core):
  Device (per core): the O(B*S^2*D) pairwise weighted-L1 cost matrix
    [4, 256, 256] plus the per-element BCE log terms.
    - The outer difference w_d*(pred[i,d] - gt[j,d]) is produced on the
      TensorEngine as K=4 matmuls per d: pred/gt are split into bf16
      hi+lo pairs on the host (exact to ~2^-17 rel), and the +-w_d
      weights ride along as constant rows.  PSUM accumulates in fp32.
    - A single fused VectorEngine tensor_reduce(apply_absolute_value)
      per tile folds |.| and the sum over d.
    - BCE: ScalarEngine Ln(p) / Ln(1-p), blended with the 0/1 targets.
  Host: shard/pack inputs, exact Jonker-Volgenant assignment per sample
    on the device-produced cost (sequential, data-dependent control
    flow - unsuited to the engines), and the final tiny reduction.
"""

import numpy as np
import ml_dtypes

B, S, D = 32, 256, 10
NCORES = 8
BPC = B // NCORES  # samples per core
_W = np.array([5.0] * 8 + [1.0] * 2, dtype=np.float32)  # per-dim L1 weights

_NC_CACHE = {}


def _build_program():
    import concourse.tile as tile
    import concourse.mybir as mybir
    from concourse import bacc

    f32 = mybir.dt.float32
    bf16 = mybir.dt.bfloat16
    AX = mybir.AxisListType
    AL = mybir.AluOpType
    ACTF = mybir.ActivationFunctionType

    # Bacc (not bass.Bass): its compile() splits multi-sem waits so matmuls
    # satisfy the TRN2 one-wait-per-instruction codegen constraint.
    nc = bacc.Bacc("TRN2", target_bir_lowering=False, debug=False)

    # Host-packed input. PE matmul operands must sit at partition base
    # 0/32/64, so the K=4 rows live on partitions 0..3; (sample, side, dim)
    # are multiplexed along the free dim: sample s occupies [s*5120,
    # (s+1)*5120) with its lhsT block (pm) first and rhs block (gm) second,
    # each indexed d*256 + col.  One contiguous DMA per sample -> the
    # sample's matmuls carry a single DMA wait (walrus limits MM waits).
    #   pm rows: [p_hi[:,d]; p_lo[:,d]; -w_d; -w_d]   (lhsT, i on free)
    #   gm rows: [ w_d    ;  w_d    ; g_hi[:,d]; g_lo[:,d]]  (rhs, j on free)
    # => (lhsT.T @ rhs)[i, j] = w_d*(p_hi+p_lo)[i] - w_d*(g_hi+g_lo)[j]
    # Samples land on partition bases {0, 32, 64, 0(+free offset)} so the
    # input DMAs hit three SBUF port groups in parallel (a 4-partition-only
    # destination is port-serialized ~13GB/s).
    # K=8 block-diagonal pairs: each matmul computes TWO d-planes (8 lhsT
    # rows = two 4-row groups; rhs [8, 2, 128] with zero off-diagonal
    # blocks, host-packed).  Halves the PE instruction stream (80 MMs) so
    # it fits one IRAM block and shortens the start barrier.
    # Per-sample free layout: [5 pairs x 256 pm | 5 pairs x 512 gm] = 3840.
    SAMP = 15 * 256  # 3840
    S_BASE = [0, 32, 64, 0]
    S_FOFF = [0, 0, 0, SAMP]
    pg_d = nc.dram_tensor("pg", [BPC, 8, SAMP], bf16, kind="ExternalInput").ap()
    pv_d = nc.dram_tensor("pv", [128, 8], f32, kind="ExternalInput").ap()
    tv_d = nc.dram_tensor("tv", [128, 8], f32, kind="ExternalInput").ap()
    cost_d = nc.dram_tensor("cost", [BPC, S, S], f32, kind="ExternalOutput").ap()
    bce_d = nc.dram_tensor("bce", [128, 8], f32, kind="ExternalOutput").ap()

    with tile.TileContext(nc) as tc:
        with (
            tc.tile_pool(name="inp", bufs=1) as inp,
            tc.tile_pool(name="ps", bufs=2, space="PSUM") as psp,
            tc.tile_pool(name="outp", bufs=4) as outp,
            tc.tile_pool(name="bcep", bufs=1) as bcep,
        ):
            # Input DMAs: one chunk per sample, sample 0 first (it gates the
            # first matmuls; measured best all on the sync queue).
            pg_sb = inp.tile([72, 2 * SAMP], bf16, tag="pg", name="pg_sb")
            for s in range(BPC):
                b, f = S_BASE[s], S_FOFF[s]
                if s == 0:
                    # Sample 0 gates the first matmuls: land the (it=0, jh=0)
                    # column halves first so unit 0 can start sooner.
                    dst = pg_sb[b : b + 8, f : f + SAMP].rearrange(
                        "p (k c) -> p k c", c=S
                    )
                    src = pg_d[s].rearrange("p (k c) -> p k c", c=S)
                    nc.sync.dma_start(dst[:, :, 0:128], src[:, :, 0:128])
                    nc.sync.dma_start(dst[:, :, 128:256], src[:, :, 128:256])
                else:
                    nc.sync.dma_start(
                        pg_sb[b : b + 8, f : f + SAMP],
                        pg_d[s],
                    )
            pv_sb = bcep.tile([128, 8], f32, tag="pv")
            tv_sb = bcep.tile([128, 8], f32, tag="tv")
            nc.scalar.dma_start(pv_sb[:], pv_d[:])
            nc.scalar.dma_start(tv_sb[:], tv_d[:])

            # ---- cost matrix: 16 units of [128 i, 10 d, 128 j] ----
            for s in range(BPC):
                for it in range(2):
                    for jh in range(2):
                        ps = psp.tile([128, 10, 128], f32, tag="unit")
                        b, f = S_BASE[s], S_FOFF[s]
                        for t in range(5):
                            fp = f + t * 256
                            fg = f + 5 * 256 + t * 512
                            rhs = pg_sb[b : b + 8, fg : fg + 512].rearrange(
                                "p (u c) -> p u c", c=256
                            )
                            nc.tensor.matmul(
                                ps[:, 2 * t : 2 * t + 2, :],
                                pg_sb[b : b + 8, fp + 128 * it : fp + 128 * (it + 1)],
                                rhs[:, :, 128 * jh : 128 * (jh + 1)],
                                start=True,
                                stop=True,
                            )
                        csb = outp.tile([128, 128], f32, tag="csb")
                        nc.vector.tensor_reduce(
                            csb[:],
                            ps[:].rearrange("p d j -> p j d"),
                            axis=AX.X,
                            op=AL.add,
                            apply_absolute_value=True,
                        )
                        nc.sync.dma_start(
                            cost_d[s, 128 * it : 128 * (it + 1), 128 * jh : 128 * (jh + 1)],
                            csb[:],
                        )

            # ---- BCE log terms (tiny): term = t*ln(p) + (1-t)*ln(1-p) ----
            lnp = bcep.tile([128, 8], f32, tag="lnp")
            ln1p = bcep.tile([128, 8], f32, tag="ln1p")
            nc.scalar.activation(lnp[:], pv_sb[:], ACTF.Ln)
            nc.scalar.activation(ln1p[:], pv_sb[:], ACTF.Ln, scale=-1.0, bias=1.0)
            dif = bcep.tile([128, 8], f32, tag="dif")
            nc.vector.tensor_tensor(dif[:], lnp[:], ln1p[:], op=AL.subtract)
            prod = bcep.tile([128, 8], f32, tag="prod")
            nc.vector.tensor_tensor(prod[:], dif[:], tv_sb[:], op=AL.mult)
            term = bcep.tile([128, 8], f32, tag="term")
            nc.vector.tensor_tensor(term[:], prod[:], ln1p[:], op=AL.add)
            nc.sync.dma_start(bce_d[:], term[:])
    nc.compile()
    return nc


def _get_nc():
    if "nc" not in _NC_CACHE:
        _NC_CACHE["nc"] = _build_program()
    return _NC_CACHE["nc"]


def _pack_core(ps_c, pv_c, tg_c):
    """Build the per-core input map from fp32 slices [BPC,S,10/1/11]."""
    bf = ml_dtypes.bfloat16
    gt_c = tg_c[..., :10]
    p_hi = ps_c.astype(bf)
    p_lo = (ps_c - p_hi.astype(np.float32)).astype(bf)
    g_hi = gt_c.astype(bf)
    g_lo = (gt_c - g_hi.astype(np.float32)).astype(bf)
    w_b = _W.astype(bf)

    # K=8 pair layout: rows 0-3 = dim 2t, rows 4-7 = dim 2t+1.
    # pm block: [8, 5 pairs, 256 i];  gm block: [8, 5 pairs, 2, 256 j]
    # with the off-diagonal (row-group, j-block) combinations left zero.
    wf = w_b.astype(np.float32)
    pm = np.zeros((BPC, 8, 5, S), dtype=bf)
    gm = np.zeros((BPC, 8, 5, 2, S), dtype=bf)
    for t in range(5):
        for h in range(2):  # h=0 -> rows 0-3 (dim 2t), h=1 -> rows 4-7 (dim 2t+1)
            d = 2 * t + h
            r = 4 * h
            pm[:, r + 0, t] = p_hi[:, :, d]
            pm[:, r + 1, t] = p_lo[:, :, d]
            pm[:, r + 2, t] = bf(-wf[d])
            pm[:, r + 3, t] = bf(-wf[d])
            gm[:, r + 0, t, h] = bf(wf[d])
            gm[:, r + 1, t, h] = bf(wf[d])
            gm[:, r + 2, t, h] = g_hi[:, :, d]
            gm[:, r + 3, t, h] = g_lo[:, :, d]
    pg = np.concatenate(
        [pm.reshape(BPC, 8, 5 * S), gm.reshape(BPC, 8, 10 * S)], axis=2
    )
    pg = np.ascontiguousarray(pg)

    pv = np.ascontiguousarray(pv_c.reshape(1024).reshape(128, 8))
    tv = np.ascontiguousarray(tg_c[..., 10].reshape(1024).reshape(128, 8))
    return {"pg": pg, "pv": pv, "tv": tv}


def _lsa(cost):
    """Rectangular linear sum assignment (Jonker-Volgenant shortest augmenting
    path), equivalent to scipy.optimize.linear_sum_assignment."""
    cost = np.asarray(cost, dtype=np.float64)
    transposed = cost.shape[0] > cost.shape[1]
    if transposed:
        cost = cost.T
    nr, ncc = cost.shape
    u = np.zeros(nr)
    v = np.zeros(ncc)
    path = np.full(ncc, -1, dtype=np.int64)
    col4row = np.full(nr, -1, dtype=np.int64)
    row4col = np.full(ncc, -1, dtype=np.int64)
    for cur_row in range(nr):
        min_val = 0.0
        i = cur_row
        remaining = np.arange(ncc)
        sp = np.full(ncc, np.inf)
        SR = np.zeros(nr, dtype=bool)
        SC = np.zeros(ncc, dtype=bool)
        sink = -1
        while sink == -1:
            SR[i] = True
            r = min_val + cost[i, remaining] - u[i] - v[remaining]
            better = r < sp[remaining]
            idx_b = remaining[better]
            sp[idx_b] = r[better]
            path[idx_b] = i
            k = int(np.argmin(sp[remaining]))
            j = int(remaining[k])
            min_val = float(sp[j])
            if row4col[j] == -1:
                sink = j
            else:
                i = int(row4col[j])
                SC[j] = True
                remaining = np.delete(remaining, k)
        u[cur_row] += min_val
        others = SR.copy()
        others[cur_row] = False
        rr = np.where(others)[0]
        u[rr] += min_val - sp[col4row[rr]]
        cc = np.where(SC)[0]
        v[cc] += sp[cc] - min_val
        j = sink
        while True:
            i = int(path[j])
            row4col[j] = i
            col4row[i], j = j, int(col4row[i])
            if i == cur_row:
                break
    if transposed:
        rows = col4row
        cols = np.arange(nr)
        order = np.argsort(rows)
        return rows[order], cols[order]
    return np.arange(nr), col4row


def _assemble_loss(pred_strokes, targets, cost, bce_terms):
    """Host: per-sample Hungarian on the device cost + final reduction."""
    gt = targets[..., :10]
    gval = targets[..., 10]
    total = 0.0
    for b in range(B):
        bce_b = -float(
            bce_terms[b // BPC].reshape(1024)[
                (b % BPC) * 256 : (b % BPC + 1) * 256
            ].astype(np.float64).mean()
        )
        cols = np.where(gval[b] > 0.5)[0]
        n = len(cols)
        if n == 0:
            total += bce_b
            continue
        r, c = _lsa(cost[b][:, cols])
        pi, gi = r, cols[c]
        mp = pred_strokes[b, pi].astype(np.float64)
        mg = gt[b, gi].astype(np.float64)
        coord = np.abs(mp[:, :8] - mg[:, :8]).sum() / max(8 * n, 1)
        width = np.abs(mp[:, 8:10] - mg[:, 8:10]).sum() / max(2 * n, 1)
        total += 5.0 * coord + width + bce_b
    return np.float32(total / B)


def kernel(pred_strokes, pred_validity, targets):
    from concourse.bass_utils import run_bass_kernel_spmd

    pred_strokes = np.asarray(pred_strokes, dtype=np.float32)
    pred_validity = np.asarray(pred_validity, dtype=np.float32)
    targets = np.asarray(targets, dtype=np.float32)

    in_maps = [
        _pack_core(
            pred_strokes[c * BPC : (c + 1) * BPC],
            pred_validity[c * BPC : (c + 1) * BPC],
            targets[c * BPC : (c + 1) * BPC],
        )
        for c in range(NCORES)
    ]

    nc = _get_nc()
    res = run_bass_kernel_spmd(nc, in_maps, list(range(NCORES)))
    results = res.results

    cost = np.concatenate([r["cost"] for r in results], axis=0)  # [32, 256, 256]
    bce_terms = [r["bce"] for r in results]  # 8 x [128, 8]

    loss = _assemble_loss(pred_strokes, targets, cost, bce_terms)
    return np.asarray(loss, dtype=np.float32)


# revision 44
# speedup vs baseline: 1.1531x; 1.0079x over previous
"""DETR-style Hungarian-matching loss on 8 Trainium2 NeuronCores.

Strategy (pure data parallel, B=32 sharded 4 samples/core):
  Device (per core): the O(B*S^2*D) pairwise weighted-L1 cost matrix
    [4, 256, 256] plus the per-element BCE log terms.
    - The outer difference w_d*(pred[i,d] - gt[j,d]) is produced on the
      TensorEngine as K=4 matmuls per d: pred/gt are split into bf16
      hi+lo pairs on the host (exact to ~2^-17 rel), and the +-w_d
      weights ride along as constant rows.  PSUM accumulates in fp32.
    - A single fused VectorEngine tensor_reduce(apply_absolute_value)
      per tile folds |.| and the sum over d.
    - BCE: ScalarEngine Ln(p) / Ln(1-p), blended with the 0/1 targets.
  Host: shard/pack inputs, exact Jonker-Volgenant assignment per sample
    on the device-produced cost (sequential, data-dependent control
    flow - unsuited to the engines), and the final tiny reduction.
"""

import numpy as np
import ml_dtypes

B, S, D = 32, 256, 10
NCORES = 8
BPC = B // NCORES  # samples per core
_W = np.array([5.0] * 8 + [1.0] * 2, dtype=np.float32)  # per-dim L1 weights

_NC_CACHE = {}


def _build_program():
    import concourse.tile as tile
    import concourse.mybir as mybir
    from concourse import bacc

    f32 = mybir.dt.float32
    bf16 = mybir.dt.bfloat16
    AX = mybir.AxisListType
    AL = mybir.AluOpType
    ACTF = mybir.ActivationFunctionType

    # Bacc (not bass.Bass): its compile() splits multi-sem waits so matmuls
    # satisfy the TRN2 one-wait-per-instruction codegen constraint.
    nc = bacc.Bacc("TRN2", target_bir_lowering=False, debug=False)

    # Host-packed input. PE matmul operands must sit at partition base
    # 0/32/64, so the K=4 rows live on partitions 0..3; (sample, side, dim)
    # are multiplexed along the free dim: sample s occupies [s*5120,
    # (s+1)*5120) with its lhsT block (pm) first and rhs block (gm) second,
    # each indexed d*256 + col.  One contiguous DMA per sample -> the
    # sample's matmuls carry a single DMA wait (walrus limits MM waits).
    #   pm rows: [p_hi[:,d]; p_lo[:,d]; -w_d; -w_d]   (lhsT, i on free)
    #   gm rows: [ w_d    ;  w_d    ; g_hi[:,d]; g_lo[:,d]]  (rhs, j on free)
    # => (lhsT.T @ rhs)[i, j] = w_d*(p_hi+p_lo)[i] - w_d*(g_hi+g_lo)[j]
    # Samples land on partition bases {0, 32, 64, 0(+free offset)} so the
    # input DMAs hit three SBUF port groups in parallel (a 4-partition-only
    # destination is port-serialized ~13GB/s).
    # K=8 block-diagonal pairs: each matmul computes TWO d-planes (8 lhsT
    # rows = two 4-row groups; rhs [8, 2, 128] with zero off-diagonal
    # blocks, host-packed).  Halves the PE instruction stream (80 MMs) so
    # it fits one IRAM block and shortens the start barrier.
    # Per-sample free layout: [5 pairs x 256 pm | 5 pairs x 512 gm] = 3840.
    SAMP = 15 * 256  # 3840
    S_BASE = [0, 32, 64, 0]
    S_FOFF = [0, 0, 0, SAMP]
    pg_d = nc.dram_tensor("pg", [BPC, 8, SAMP], bf16, kind="ExternalInput").ap()
    pv_d = nc.dram_tensor("pv", [128, 8], f32, kind="ExternalInput").ap()
    tv_d = nc.dram_tensor("tv", [128, 8], f32, kind="ExternalInput").ap()
    cost_d = nc.dram_tensor("cost", [BPC, S, S], f32, kind="ExternalOutput").ap()
    bce_d = nc.dram_tensor("bce", [128, 8], f32, kind="ExternalOutput").ap()

    with tile.TileContext(nc) as tc:
        with (
            tc.tile_pool(name="inp", bufs=1) as inp,
            tc.tile_pool(name="ps", bufs=2, space="PSUM") as psp,
            tc.tile_pool(name="outp", bufs=4) as outp,
            tc.tile_pool(name="bcep", bufs=1) as bcep,
        ):
            # Input DMAs: one chunk per sample, sample 0 first (it gates the
            # first matmuls; measured best all on the sync queue).
            pg_sb = inp.tile([72, 2 * SAMP], bf16, tag="pg", name="pg_sb")
            for s in range(BPC):
                b, f = S_BASE[s], S_FOFF[s]
                if s == 0:
                    # Sample 0 gates the first matmuls: land the (it=0, jh=0)
                    # column halves first so unit 0 can start sooner.
                    dst = pg_sb[b : b + 8, f : f + SAMP].rearrange(
                        "p (k c) -> p k c", c=S
                    )
                    src = pg_d[s].rearrange("p (k c) -> p k c", c=S)
                    nc.sync.dma_start(dst[:, :, 0:128], src[:, :, 0:128])
                    nc.sync.dma_start(dst[:, :, 128:256], src[:, :, 128:256])
                else:
                    nc.sync.dma_start(
                        pg_sb[b : b + 8, f : f + SAMP],
                        pg_d[s],
                    )
            pv_sb = bcep.tile([128, 8], f32, tag="pv")
            tv_sb = bcep.tile([128, 8], f32, tag="tv")
            nc.scalar.dma_start(pv_sb[:], pv_d[:])
            nc.scalar.dma_start(tv_sb[:], tv_d[:])

            # ---- cost matrix: 16 units of [128 i, 10 d, 128 j] ----
            for s in range(BPC):
                for it in range(2):
                    for jh in range(2):
                        ps = psp.tile([128, 10, 128], f32, tag="unit")
                        b, f = S_BASE[s], S_FOFF[s]
                        for t in range(5):
                            fp = f + t * 256
                            fg = f + 5 * 256 + t * 512
                            rhs = pg_sb[b : b + 8, fg : fg + 512].rearrange(
                                "p (u c) -> p u c", c=256
                            )
                            nc.tensor.matmul(
                                ps[:, 2 * t : 2 * t + 2, :],
                                pg_sb[b : b + 8, fp + 128 * it : fp + 128 * (it + 1)],
                                rhs[:, :, 128 * jh : 128 * (jh + 1)],
                                start=True,
                                stop=True,
                            )
                        csb = outp.tile([128, 128], f32, tag="csb")
                        nc.vector.tensor_reduce(
                            csb[:],
                            ps[:].rearrange("p d j -> p j d"),
                            axis=AX.X,
                            op=AL.add,
                            apply_absolute_value=True,
                        )
                        nc.sync.dma_start(
                            cost_d[s, 128 * it : 128 * (it + 1), 128 * jh : 128 * (jh + 1)],
                            csb[:],
                        )

            # ---- BCE log terms (tiny): term = t*ln(p) + (1-t)*ln(1-p) ----
            lnp = bcep.tile([128, 8], f32, tag="lnp")
            ln1p = bcep.tile([128, 8], f32, tag="ln1p")
            nc.scalar.activation(lnp[:], pv_sb[:], ACTF.Ln)
            nc.scalar.activation(ln1p[:], pv_sb[:], ACTF.Ln, scale=-1.0, bias=1.0)
            dif = bcep.tile([128, 8], f32, tag="dif")
            nc.vector.tensor_tensor(dif[:], lnp[:], ln1p[:], op=AL.subtract)
            prod = bcep.tile([128, 8], f32, tag="prod")
            nc.vector.tensor_tensor(prod[:], dif[:], tv_sb[:], op=AL.mult)
            term = bcep.tile([128, 8], f32, tag="term")
            nc.vector.tensor_tensor(term[:], prod[:], ln1p[:], op=AL.add)
            nc.sync.dma_start(bce_d[:], term[:])
    nc.compile()
    return nc


def _get_nc():
    if "nc" not in _NC_CACHE:
        _NC_CACHE["nc"] = _build_program()
    return _NC_CACHE["nc"]


def _pack_core(ps_c, pv_c, tg_c):
    """Build the per-core input map from fp32 slices [BPC,S,10/1/11]."""
    bf = ml_dtypes.bfloat16
    gt_c = tg_c[..., :10]
    p_hi = ps_c.astype(bf)
    p_lo = (ps_c - p_hi.astype(np.float32)).astype(bf)
    g_hi = gt_c.astype(bf)
    g_lo = (gt_c - g_hi.astype(np.float32)).astype(bf)
    w_b = _W.astype(bf)

    # K=8 pair layout: rows 0-3 = dim 2t, rows 4-7 = dim 2t+1.
    # pm block: [8, 5 pairs, 256 i];  gm block: [8, 5 pairs, 2, 256 j]
    # with the off-diagonal (row-group, j-block) combinations left zero.
    wf = w_b.astype(np.float32)
    pm = np.zeros((BPC, 8, 5, S), dtype=bf)
    gm = np.zeros((BPC, 8, 5, 2, S), dtype=bf)
    for t in range(5):
        for h in range(2):  # h=0 -> rows 0-3 (dim 2t), h=1 -> rows 4-7 (dim 2t+1)
            d = 2 * t + h
            r = 4 * h
            pm[:, r + 0, t] = p_hi[:, :, d]
            pm[:, r + 1, t] = p_lo[:, :, d]
            pm[:, r + 2, t] = bf(-wf[d])
            pm[:, r + 3, t] = bf(-wf[d])
            gm[:, r + 0, t, h] = bf(wf[d])
            gm[:, r + 1, t, h] = bf(wf[d])
            gm[:, r + 2, t, h] = g_hi[:, :, d]
            gm[:, r + 3, t, h] = g_lo[:, :, d]
    pg = np.concatenate(
        [pm.reshape(BPC, 8, 5 * S), gm.reshape(BPC, 8, 10 * S)], axis=2
    )
    pg = np.ascontiguousarray(pg)

    pv = np.ascontiguousarray(pv_c.reshape(1024).reshape(128, 8))
    tv = np.ascontiguousarray(tg_c[..., 10].reshape(1024).reshape(128, 8))
    return {"pg": pg, "pv": pv, "tv": tv}


def _lsa(cost):
    """Rectangular linear sum assignment (Jonker-Volgenant shortest augmenting
    path), equivalent to scipy.optimize.linear_sum_assignment."""
    cost = np.asarray(cost, dtype=np.float64)
    transposed = cost.shape[0] > cost.shape[1]
    if transposed:
        cost = cost.T
    nr, ncc = cost.shape
    u = np.zeros(nr)
    v = np.zeros(ncc)
    path = np.full(ncc, -1, dtype=np.int64)
    col4row = np.full(nr, -1, dtype=np.int64)
    row4col = np.full(ncc, -1, dtype=np.int64)
    for cur_row in range(nr):
        min_val = 0.0
        i = cur_row
        remaining = np.arange(ncc)
        sp = np.full(ncc, np.inf)
        SR = np.zeros(nr, dtype=bool)
        SC = np.zeros(ncc, dtype=bool)
        sink = -1
        while sink == -1:
            SR[i] = True
            r = min_val + cost[i, remaining] - u[i] - v[remaining]
            better = r < sp[remaining]
            idx_b = remaining[better]
            sp[idx_b] = r[better]
            path[idx_b] = i
            k = int(np.argmin(sp[remaining]))
            j = int(remaining[k])
            min_val = float(sp[j])
            if row4col[j] == -1:
                sink = j
            else:
                i = int(row4col[j])
                SC[j] = True
                remaining = np.delete(remaining, k)
        u[cur_row] += min_val
        others = SR.copy()
        others[cur_row] = False
        rr = np.where(others)[0]
        u[rr] += min_val - sp[col4row[rr]]
        cc = np.where(SC)[0]
        v[cc] += sp[cc] - min_val
        j = sink
        while True:
            i = int(path[j])
            row4col[j] = i
            col4row[i], j = j, int(col4row[i])
            if i == cur_row:
                break
    if transposed:
        rows = col4row
        cols = np.arange(nr)
        order = np.argsort(rows)
        return rows[order], cols[order]
    return np.arange(nr), col4row


def _assemble_loss(pred_strokes, targets, cost, bce_terms):
    """Host: per-sample Hungarian on the device cost + final reduction."""
    gt = targets[..., :10]
    gval = targets[..., 10]
    total = 0.0
    for b in range(B):
        bce_b = -float(
            bce_terms[b // BPC].reshape(1024)[
                (b % BPC) * 256 : (b % BPC + 1) * 256
            ].astype(np.float64).mean()
        )
        cols = np.where(gval[b] > 0.5)[0]
        n = len(cols)
        if n == 0:
            total += bce_b
            continue
        r, c = _lsa(cost[b][:, cols])
        pi, gi = r, cols[c]
        mp = pred_strokes[b, pi].astype(np.float64)
        mg = gt[b, gi].astype(np.float64)
        coord = np.abs(mp[:, :8] - mg[:, :8]).sum() / max(8 * n, 1)
        width = np.abs(mp[:, 8:10] - mg[:, 8:10]).sum() / max(2 * n, 1)
        total += 5.0 * coord + width + bce_b
    return np.float32(total / B)


def kernel(pred_strokes, pred_validity, targets):
    from concourse.bass_utils import run_bass_kernel_spmd

    pred_strokes = np.asarray(pred_strokes, dtype=np.float32)
    pred_validity = np.asarray(pred_validity, dtype=np.float32)
    targets = np.asarray(targets, dtype=np.float32)

    in_maps = [
        _pack_core(
            pred_strokes[c * BPC : (c + 1) * BPC],
            pred_validity[c * BPC : (c + 1) * BPC],
            targets[c * BPC : (c + 1) * BPC],
        )
        for c in range(NCORES)
    ]

    nc = _get_nc()
    res = run_bass_kernel_spmd(nc, in_maps, list(range(NCORES)))
    results = res.results

    cost = np.concatenate([r["cost"] for r in results], axis=0)  # [32, 256, 256]
    bce_terms = [r["bce"] for r in results]  # 8 x [128, 8]

    loss = _assemble_loss(pred_strokes, targets, cost, bce_terms)
    return np.asarray(loss, dtype=np.float32)
